# revision 4
# baseline (speedup 1.0000x reference)
"""Trainium2 Bass kernel for Luong-attention (nn_Attention_4174708212176).

out[b] = softmax(dec[b] @ (enc[b] @ W)^T) @ enc[b],  b = 0..7, one batch per core.

Precision scheme (tolerance is rel 2e-2; this sits ~5e-3):
- All three matmuls in plain fp16 (PE upconverts to e10m11, fp32 PSUM accum).
  Logit noise eps ~ 0.3 from fp16 rounding of dec/ep shifts soft rows only;
  P is near-one-hot so the bulk of the output is unaffected.
- Softmax fp32 on DVE (max, negated) + ACT (exp with accumulated row sums);
  1/sum is folded into the final PSUM->SBUF copy via activation(Copy, scale).
- P^T for M3 via PE transposes (fp16), 4 per PSUM bank, copied out on ACT/DVE.

The PE stream is software-pipelined: transposes+M3 of tile t-1 are emitted
after tile t's logits matmuls so the PE never waits on the softmax engines.

Layouts are prepared host-side: each core receives one packed fp16 tensor
(W, encT sc-major, decT, enc natural), DMA'd in segments so M1 starts as
soon as W and the first encT chunk arrive.
"""
import contextlib
import numpy as np

import concourse.bass as bass
import concourse.tile as tile
from concourse import bacc, mybir
from concourse.bass_utils import run_bass_kernel_spmd
from concourse.masks import make_identity

B, S, T, E, D = 8, 2048, 2048, 512, 512
P = 128
DO = D // P      # 4  d-tiles
EO = E // P      # 4  e-tiles
SO = S // P      # 16 s-tiles
TO = T // P      # 16 t-tiles
SC = S // 512    # 4  512-wide s-chunks
NCORES = 8

# packed free-dim offsets (fp16 elements per partition)
OFF_WH = 0                   # W        [4, 512]
OFF_ETH = OFF_WH + EO * D    # encT     [4, 2048] sc-major
OFF_DTH = OFF_ETH + EO * S   # decT     [4, 2048]
OFF_EN = OFF_DTH + DO * T    # enc natural [16, 512]
FREE = OFF_EN + SO * E

SEGS = [  # (name, offset, width)
    ("w", OFF_WH, EO * D),
    ("eth", OFF_ETH, EO * S),
    ("dt", OFF_DTH, DO * T),
    ("en", OFF_EN, SO * E),
]

_compiled_nc = {}


def _build(reps=1):
    nc = bacc.Bacc()
    x_in = nc.declare_dram_parameter("x", [P, FREE], mybir.dt.float16, isOutput=False)
    out_d = nc.declare_dram_parameter("out", [T, E], mybir.dt.float32, isOutput=True)

    with tile.TileContext(nc) as tc:
        with tc.tile_pool(name="const", bufs=1) as cpool, \
             tc.tile_pool(name="ep", bufs=1) as eppool, \
             tc.tile_pool(name="work", bufs=3) as wpool, \
             tc.tile_pool(name="lbuf", bufs=2) as lpool, \
             tc.tile_pool(name="stat", bufs=4) as spool, \
             tc.tile_pool(name="psA", bufs=5, space="PSUM") as psA, \
             tc.tile_pool(name="psB", bufs=2, space="PSUM") as psB, \
             tc.tile_pool(name="psC", bufs=1, space="PSUM") as psC:

            ident = cpool.tile([P, P], mybir.dt.float16)
            make_identity(nc, ident[:])

            _ENGS = (mybir.EngineType.PE, mybir.EngineType.Activation,
                     mybir.EngineType.DVE, mybir.EngineType.SP,
                     mybir.EngineType.Pool)
            loop_ctx = (tc.For_i(0, reps, 1, hint_engines=_ENGS)
                        if reps > 1 else contextlib.nullcontext())
            with loop_ctx:
                _body(nc, tc, cpool, eppool, wpool, lpool, spool,
                      psA, psB, psC, x_in, out_d, ident)

    nc.compile()
    return nc


def _body(nc, tc, cpool, eppool, wpool, lpool, spool, psA, psB, psC,
          x_in, out_d, ident):
    seg = {}
    for name, off, width in SEGS:
        seg[name] = cpool.tile([P, width], mybir.dt.float16, tag=f"seg_{name}",
                               name=f"seg_{name}")
    segd = dict((n, (o, w)) for n, o, w in SEGS)
    nc.sync.dma_start(seg["w"][:], x_in.ap()[:, segd["w"][0]:segd["w"][0] + segd["w"][1]])
    # per-sc chunks of encT so M1's first accumulation group never waits
    for sc in range(SC):
        off, width = segd["eth"]
        w4 = width // SC
        nc.sync.dma_start(seg["eth"][:, sc * w4:(sc + 1) * w4],
                          x_in.ap()[:, off + sc * w4:off + (sc + 1) * w4])
    for name in ("dt", "en"):
        off, width = segd[name]
        nc.sync.dma_start(seg[name][:], x_in.ap()[:, off:off + width])

    def wh(eo, do):  # W tile [128, 128] (lhsT for M1)
        o = eo * D + do * P
        return seg["w"][:, o:o + P]

    def eth(eo, sc):  # encT chunk [128, 512] (rhs for M1), sc-major
        o = sc * 4 * 512 + eo * 512
        return seg["eth"][:, o:o + 512]

    def dth(do, tt):  # decT tile [128, 128] (lhsT for M2)
        o = do * T + tt * P
        return seg["dt"][:, o:o + P]

    def encn(st):  # enc natural tile [128, 512] (rhs for M3)
        o = st * E
        return seg["en"][:, o:o + 512]

    # ---- M1: epT[d, s] = sum_e W[e, d] * encT[e, s], fp16
    eph = eppool.tile([P, DO * S], mybir.dt.float16)  # [128, 4*2048]
    for do in range(DO):
        pss = [psA.tile([P, 512], mybir.dt.float32, tag="ps_l", name=f"m1_{do}_{sc}")
               for sc in range(SC)]
        for eo in range(EO):
            for sc in range(SC):
                nc.tensor.matmul(pss[sc][:], wh(eo, do), eth(eo, sc),
                                 start=(eo == 0), stop=(eo == EO - 1),
                                 skip_group_check=True)
        for sc in range(SC):
            dst = slice(do * S + sc * 512, do * S + sc * 512 + 512)
            if sc % 2 == 0:
                nc.scalar.copy(eph[:, dst], pss[sc][:])
            else:
                nc.vector.tensor_copy(eph[:, dst], pss[sc][:])

    def ephc(do, sc):
        o = do * S + sc * 512
        return eph[:, o:o + 512]

    # ---- per t-tile phases. The PE stream is pipelined 2 deep: iteration tt
    # emits [M2(tt) | tr(tt-2)+M3(tt-2) | softmax-engine-ops(tt)] so the
    # softmax chain of tile tt has two full M2+tr+M3 spans to complete before
    # the PE needs p_sb(tt). Emission order also keeps the strict-FIFO ACT/DVE
    # queues unblocked: pt copies (needed by M3 this iteration) come before
    # the softmax ops of tt.
    def emit_m2(tt):
        pss = [psA.tile([P, 512], mybir.dt.float32, tag="ps_l", name=f"m2_{tt}_{sc}")
               for sc in range(SC)]
        for do in range(DO):
            for sc in range(SC):
                nc.tensor.matmul(pss[sc][:], dth(do, tt), ephc(do, sc),
                                 start=(do == 0), stop=(do == DO - 1),
                                 skip_group_check=True)
        return pss

    def emit_softmax(tt, pss):
        l_sb = lpool.tile([P, S], mybir.dt.float32, name=f"l{tt}", tag="l")
        for sc in range(SC):
            dst = slice(sc * 512, sc * 512 + 512)
            if sc % 2 == 0:
                nc.scalar.copy(l_sb[:, dst], pss[sc][:])
            else:
                nc.vector.tensor_copy(l_sb[:, dst], pss[sc][:])

        negmax = spool.tile([P, 1], mybir.dt.float32, name=f"negmax{tt}", tag="negmax")
        nc.vector.tensor_reduce(negmax[:], l_sb[:], axis=mybir.AxisListType.X,
                                op=mybir.AluOpType.max, negate=True)

        p_sb = wpool.tile([P, S], mybir.dt.float16, name=f"p{tt}", tag="p")
        ssum = spool.tile([P, 1], mybir.dt.float32, name=f"ssum{tt}", tag="ssum")
        nc.scalar.activation(p_sb[:], l_sb[:],
                             mybir.ActivationFunctionType.Exp,
                             bias=negmax[:], scale=1.0,
                             accum_out=ssum[:])
        recip = spool.tile([P, 1], mybir.dt.float32, name=f"recip{tt}", tag="recip")
        nc.vector.reciprocal(recip[:], ssum[:])
        return p_sb, recip

    def emit_tr_m3(tt, p_sb, recip):
        # transpose P [128t, 2048s] -> PT tiles [128s, 128t], batched 4 per PSUM
        pt_sb = wpool.tile([P, SO * P], mybir.dt.float16, name=f"pt{tt}", tag="pt")
        for q in range(SO // 4):
            ps_tr = psB.tile([P, 512], mybir.dt.float16, tag="ps_tr", name=f"tr{tt}_{q}")
            for j in range(4):
                st = q * 4 + j
                nc.tensor.transpose(ps_tr[:, j * P:(j + 1) * P],
                                    p_sb[:, st * P:(st + 1) * P], ident[:])
            dst = slice(q * 512, (q + 1) * 512)
            if q % 2 == 0:
                nc.scalar.copy(pt_sb[:, dst], ps_tr[:])
            else:
                nc.vector.tensor_copy(pt_sb[:, dst], ps_tr[:])

        # M3: out[t, e] = sum_s PT[s, t]^T * enc_n[s, e]
        ops = psC.tile([P, E], mybir.dt.float32, tag="ps_out", name=f"m3_{tt}")
        for st in range(SO):
            nc.tensor.matmul(ops[:], pt_sb[:, st * P:(st + 1) * P], encn(st),
                             start=(st == 0), stop=(st == SO - 1))
        return ops

    def emit_out(tt, ops, recip):
        # 1/rowsum folded into the PSUM->SBUF copy; emitted after softmax(tt')
        # so the M3-dependent scale never FIFO-blocks the l_sb copies on DVE.
        out_sb = wpool.tile([P, E], mybir.dt.float32, name=f"o{tt}", tag="o")
        nc.vector.tensor_scalar_mul(out_sb[:], ops[:], recip[:])
        nc.sync.dma_start(out_d.ap()[tt * P:(tt + 1) * P, :], out_sb[:])

    pending = []   # tiles whose tr/M3 is not yet emitted: (tt, p_sb, recip)
    for tt in range(TO):
        pss = emit_m2(tt)
        fin = None
        if len(pending) >= 2:
            ott, op_sb, orecip = pending.pop(0)
            ops = emit_tr_m3(ott, op_sb, orecip)
            fin = (ott, ops, orecip)
        p_sb, recip = emit_softmax(tt, pss)
        pending.append((tt, p_sb, recip))
        if fin is not None:
            emit_out(*fin)
    for ott, op_sb, orecip in pending:
        ops = emit_tr_m3(ott, op_sb, orecip)
        emit_out(ott, ops, orecip)


def _part(x, ko):
    """[K, F] -> [128, ko, F] -> [128, ko*F] flat, partition = k % 128."""
    kf = x.reshape(ko, P, -1).transpose(1, 0, 2)
    return np.ascontiguousarray(kf.reshape(P, -1))


def _f16(x):
    return x.astype(np.float16)


def _pack_core(enc_b, dec_b, wseg):
    decT = np.ascontiguousarray(dec_b.T)          # [512, 2048]
    encT = np.ascontiguousarray(enc_b.T)          # [512, 2048]
    dth = _f16(decT)
    eth = _f16(encT)
    en = _f16(enc_b)                              # [2048, 512]
    def scmajor(x):  # [128, EO*S] with [eo][sc][512] -> [sc][eo][512]
        v = _part(x, EO).reshape(P, EO, SC, 512)
        return np.ascontiguousarray(v.transpose(0, 2, 1, 3)).reshape(P, -1)

    segs = [
        wseg,
        scmajor(eth),
        _part(dth, DO),
        _part(en, SO),
    ]
    return np.concatenate(segs, axis=1)


def _make_wseg(W):
    return _part(_f16(W), EO)


def _pack_inputs(enc, dec, W):
    wseg = _make_wseg(W)
    return [{"x": _pack_core(enc[b], dec[b], wseg)} for b in range(NCORES)]


def kernel(enc_hidden_states, dec_hidden_states, W_att):
    enc = np.asarray(enc_hidden_states, np.float32)
    dec = np.asarray(dec_hidden_states, np.float32)
    W = np.asarray(W_att, np.float32)

    in_maps = _pack_inputs(enc, dec, W)

    if 1 not in _compiled_nc:
        _compiled_nc[1] = _build(1)

    res = run_bass_kernel_spmd(_compiled_nc[1], in_maps, list(range(NCORES)))
    out = np.stack([res.results[b]["out"] for b in range(NCORES)], axis=0)
    return out.astype(np.float32)


if __name__ == "__main__":
    rng = np.random.default_rng(0)
    enc = rng.standard_normal((B, S, E), dtype=np.float32)
    dec = rng.standard_normal((B, T, D), dtype=np.float32)
    W = rng.standard_normal((E, D), dtype=np.float32)
    out = kernel(enc, dec, W)
    print("out", out.shape, out.dtype)
